# revision 12
# baseline (speedup 1.0000x reference)
"""Trainium2 Bass kernel for nn_CombNetHE (dual-MLP + HE-friendly soft blend).

Contract: kernel(**inputs) takes FULL unsharded fp32 inputs (numpy or jax
arrays) and returns the FULL [16384, 10] fp32 output. Internally shards the
batch across 8 NeuronCores (2048 rows each), runs a fused Tile kernel per
core, and gathers on the host.

Per-core layout:
  - x is pre-transposed/cast on host to xT [D_IN, M] bf16 so layer 1 can run
    with W1 as the stationary operand and xT as the 512-wide moving operand,
    producing h TRANSPOSED (hT[dh, m]) in PSUM. That makes hT directly usable
    as the stationary operand of layer 2 (logits[m, 10]) with no on-chip
    transposes anywhere.
  - relu(h + b1) + bf16 cast is a single ScalarE activation per [128, 512]
    tile, PSUM -> SBUF.
  - softmax / comp_max_tau / blend run in fp32 on DVE+ACT, batched over the
    whole 512-row block ([128 partitions x 4 chunks]).
"""

import sys

sys.path.insert(0, "/opt/trn_rl_repo")

from contextlib import ExitStack

import ml_dtypes
import numpy as np

import concourse.bass as bass
import concourse.bacc as bacc
import concourse.mybir as mybir
import concourse.tile as tile
from concourse.bass_utils import run_bass_kernel_spmd

B, D_IN, D_H, C = 16384, 1024, 4096, 10
TAU, T1, T2 = 0.5, 3, 3
N_CORES = 8
M = B // N_CORES  # rows per core
M_BLK = 512  # rows processed per outer block
N_MBLK = M // M_BLK  # 4
MM = M_BLK // 128  # 4 partition-chunks per block
KC = D_IN // 128  # 8 contraction chunks (layer 1)
DC = D_H // 128  # 32 hidden chunks

F32 = mybir.dt.float32
BF16 = mybir.dt.bfloat16
Alu = mybir.AluOpType
Act = mybir.ActivationFunctionType

LAST_RESULTS = None
_BUILD_CACHE = {}


def _build_module(reps=1):
    nc = bacc.Bacc(
        "TRN2", target_bir_lowering=False, debug=False, num_devices=N_CORES
    )

    xT_d = nc.dram_tensor("xT", [D_IN, M], BF16, kind="ExternalInput")
    w1_d = {}
    w2_d = {}
    b1_d = {}
    b2_d = {}
    for n in ("o", "f"):
        w1_d[n] = nc.dram_tensor(f"w1{n}", [D_IN, D_H], BF16, kind="ExternalInput")
        w2_d[n] = nc.dram_tensor(f"w2{n}", [D_H, C], BF16, kind="ExternalInput")
        b1_d[n] = nc.dram_tensor(f"b1{n}", [D_H], F32, kind="ExternalInput")
        b2_d[n] = nc.dram_tensor(f"b2{n}", [C], F32, kind="ExternalInput")
    out_d = nc.dram_tensor("out", [M, C], F32, kind="ExternalOutput")

    with tile.TileContext(nc) as tc, ExitStack() as ctx:
        consts = ctx.enter_context(tc.tile_pool(name="consts", bufs=1))
        hpool = ctx.enter_context(tc.tile_pool(name="hpool", bufs=6))
        epool = ctx.enter_context(tc.tile_pool(name="epool", bufs=2))
        opool = ctx.enter_context(tc.tile_pool(name="opool", bufs=3))
        psum_h = ctx.enter_context(tc.tile_pool(name="psum_h", bufs=4, space="PSUM"))
        psum_o = ctx.enter_context(tc.tile_pool(name="psum_o", bufs=1, space="PSUM"))

        # ---- resident tensors -------------------------------------------
        # DMAs are emitted in consumption order so PE can start ~5us in:
        # xT(blk0), then W1o by dh-group, small tensors, W1f, xT(blk1..3).
        GS = 512  # dh elements per weight group tile
        NG = D_H // GS  # 8 groups
        DC_G = GS // 128  # dh chunks per group

        # xT: per (blk, kc) tiles [128, M_BLK] bf16, k = kc*128 + p
        xT_sb = [[None] * KC for _ in range(N_MBLK)]

        def load_x_blk(blk):
            for kc in range(KC):
                t = consts.tile(
                    [128, M_BLK], BF16, name=f"xT{blk}_{kc}", tag=f"xT{blk}_{kc}"
                )
                nc.sync.dma_start(
                    t[:],
                    xT_d.ap()[
                        kc * 128 : (kc + 1) * 128, blk * M_BLK : (blk + 1) * M_BLK
                    ],
                )
                xT_sb[blk][kc] = t

        # PE pre-warm: dummy matmuls on memset tiles run during the initial
        # weight DMA, so the clock-gate/p-state ramp burns idle time, not
        # real work. Uses a rotating ph slot; results are never read.
        warm_w = consts.tile([128, 128], BF16, name="warm_w", tag="warm_w")
        warm_x = consts.tile([128, M_BLK], BF16, name="warm_x", tag="warm_x")
        nc.vector.memset(warm_w[:], 0.0)
        nc.vector.memset(warm_x[:], 0.0)
        for _ in range(16):
            ph = psum_h.tile([128, M_BLK], F32, name="ph", tag="ph")
            nc.tensor.matmul(ph[:], lhsT=warm_w[:], rhs=warm_x[:])

        w1_sb = {n: [] for n in ("o", "f")}

        def load_w1_group(n, g):
            t = consts.tile(
                [128, KC, GS], BF16, name=f"w1{n}g{g}", tag=f"w1{n}g{g}"
            )
            nc.sync.dma_start(
                t[:],
                w1_d[n].ap()[:, g * GS : (g + 1) * GS].rearrange(
                    "(kc p) d -> p kc d", p=128
                ),
            )
            w1_sb[n].append(t)

        load_w1_group("o", 0)
        load_x_blk(0)

        w2_sb = {}
        b1_sb = {}
        b2_sb = {}
        for n in ("o", "f"):
            t = consts.tile([128, DC, C], BF16, name=f"w2{n}", tag=f"w2{n}")
            nc.sync.dma_start(
                t[:], w2_d[n].ap().rearrange("(dc p) c -> p dc c", p=128)
            )
            w2_sb[n] = t
            t = consts.tile([128, DC], F32, name=f"b1{n}", tag=f"b1{n}")
            nc.sync.dma_start(t[:], b1_d[n].ap().rearrange("(dc p) -> p dc", p=128))
            b1_sb[n] = t
            t = consts.tile([128, C], F32, name=f"b2{n}", tag=f"b2{n}")
            nc.sync.dma_start(
                t[:],
                bass.AP(tensor=b2_d[n], offset=0, ap=[[0, 128], [1, C]]),
            )
            b2_sb[n] = t
        for g in range(1, NG):
            load_w1_group("o", g)
        for g in range(NG):
            load_w1_group("f", g)

        for blk in range(1, N_MBLK):
            load_x_blk(blk)

        # ---- main loop ---------------------------------------------------
        for blk in range(N_MBLK * reps):
            blk = blk % N_MBLK
            m0 = blk * M_BLK
            probs = {}
            for n in ("o", "f"):
                # layer 1 + layer 2 fused over hidden chunks
                po = [
                    psum_o.tile([128, C], F32, name=f"po{mm}", tag=f"po{mm}")
                    for mm in range(MM)
                ]
                for dc in range(DC):
                    g, dl = dc // DC_G, dc % DC_G
                    ph = psum_h.tile([128, M_BLK], F32, name="ph", tag="ph")
                    for kc in range(KC):
                        nc.tensor.matmul(
                            ph[:],
                            lhsT=w1_sb[n][g][:, kc, dl * 128 : (dl + 1) * 128],
                            rhs=xT_sb[blk][kc][:],
                            start=(kc == 0),
                            stop=(kc == KC - 1),
                        )
                    hT = hpool.tile([128, M_BLK], BF16, name="hT", tag="hT")
                    nc.scalar.activation(
                        hT[:], ph[:], Act.Relu, bias=b1_sb[n][:, dc : dc + 1]
                    )
                    for mm in range(MM):
                        nc.tensor.matmul(
                            po[mm][:],
                            lhsT=hT[:, mm * 128 : (mm + 1) * 128],
                            rhs=w2_sb[n][:, dc, :],
                            start=(dc == 0),
                            stop=(dc == DC - 1),
                        )

                # softmax over C, batched [128, MM, C]
                z = epool.tile([128, MM, C], F32, name=f"z{n}", tag=f"z{n}")
                negmax = epool.tile([128, MM], F32, name=f"ngm{n}", tag=f"ngm{n}")
                exps = epool.tile([128, MM, C], F32, name=f"ex{n}", tag=f"ex{n}")
                sums = epool.tile([128, MM], F32, name=f"sm{n}", tag=f"sm{n}")
                rinv = epool.tile([128, MM], F32, name=f"ri{n}", tag=f"ri{n}")
                wrk = epool.tile([128, MM], F32, name=f"wk{n}", tag=f"wk{n}")
                for mm in range(MM):
                    nc.vector.tensor_tensor(
                        z[:, mm, :], po[mm][:], b2_sb[n][:], Alu.add
                    )
                for mm in range(MM):
                    nc.vector.tensor_reduce(
                        negmax[:, mm : mm + 1],
                        z[:, mm, :],
                        axis=mybir.AxisListType.X,
                        op=Alu.max,
                        negate=True,
                    )
                for mm in range(MM):
                    nc.scalar.activation(
                        exps[:, mm, :],
                        z[:, mm, :],
                        Act.Exp,
                        bias=negmax[:, mm : mm + 1],
                        accum_out=sums[:, mm : mm + 1],
                    )
                nc.vector.reciprocal(rinv[:], sums[:])
                # one Newton step: r <- r * (2 - s*r)
                nc.vector.tensor_tensor(wrk[:], sums[:], rinv[:], Alu.mult)
                nc.vector.tensor_scalar(wrk[:], wrk[:], -1.0, 2.0, Alu.mult, Alu.add)
                nc.vector.tensor_tensor(rinv[:], rinv[:], wrk[:], Alu.mult)
                pr = epool.tile([128, MM, C], F32, name=f"pr{n}", tag=f"pr{n}")
                nc.vector.tensor_tensor(
                    pr[:],
                    exps[:],
                    rinv[:, :, None].to_broadcast([128, MM, C]),
                    Alu.mult,
                )
                probs[n] = pr

            # ---- comp_max_tau on probs["o"] ------------------------------
            res = epool.tile([128, MM, C + 1], F32, name="res", tag="res")
            s4 = epool.tile([128, MM], F32, name="s4", tag="s4")
            u4 = epool.tile([128, MM], F32, name="u4", tag="u4")
            b4 = epool.tile([128, MM], F32, name="b4", tag="b4")
            a4 = epool.tile([128, MM], F32, name="a4", tag="a4")
            nc.vector.tensor_scalar(
                res[:, :, 0:C], probs["o"][:], 0.0, None, Alu.add
            )
            nc.vector.memset(res[:, :, C : C + 1], TAU)
            for i in range(T1):
                m_i = 2.0 + TAU * TAU if i == 0 else 2.0
                k_i = 2.0 / m_i
                nc.vector.tensor_tensor(res[:], res[:], res[:], Alu.mult)
                nc.vector.tensor_reduce(
                    s4[:], res[:], axis=mybir.AxisListType.X, op=Alu.add
                )
                nc.vector.tensor_scalar(u4[:], s4[:], k_i, None, Alu.mult)
                nc.vector.tensor_scalar(b4[:], u4[:], -1.0, 1.0, Alu.mult, Alu.add)
                nc.vector.tensor_scalar(
                    a4[:], u4[:], -k_i, 2.0 * k_i, Alu.mult, Alu.add
                )
                for _ in range(T2):
                    nc.vector.tensor_tensor(b4[:], b4[:], b4[:], Alu.mult)
                    nc.vector.scalar_tensor_tensor(
                        a4[:], b4[:], 1.0, a4[:], Alu.add, Alu.mult
                    )
                nc.vector.tensor_tensor(
                    res[:],
                    res[:],
                    a4[:, :, None].to_broadcast([128, MM, C + 1]),
                    Alu.mult,
                )

            # ---- blend: out = x1 + cond * (x2 - x1) ----------------------
            dd = epool.tile([128, MM, C], F32, name="dd", tag="dd")
            outt = opool.tile([128, MM, C], F32, name="outt", tag="outt")
            nc.vector.tensor_tensor(dd[:], probs["f"][:], probs["o"][:], Alu.subtract)
            nc.vector.tensor_tensor(
                dd[:],
                dd[:],
                res[:, :, C : C + 1].to_broadcast([128, MM, C]),
                Alu.mult,
            )
            nc.vector.tensor_tensor(outt[:], dd[:], probs["o"][:], Alu.add)
            nc.sync.dma_start(
                out_d.ap()[m0 : m0 + M_BLK, :].rearrange("(mm p) c -> p mm c", p=128),
                outt[:],
            )

    nc.compile()
    return nc


def _get_module():
    if "nc" not in _BUILD_CACHE:
        _BUILD_CACHE["nc"] = _build_module()
    return _BUILD_CACHE["nc"]


def kernel(x, W1o, b1o, W2o, b2o, W1f, b1f, W2f, b2f):
    import os

    x = np.asarray(x, dtype=np.float32)
    bf = ml_dtypes.bfloat16
    w1 = {
        "o": np.ascontiguousarray(np.asarray(W1o, np.float32).astype(bf)),
        "f": np.ascontiguousarray(np.asarray(W1f, np.float32).astype(bf)),
    }
    w2 = {
        "o": np.ascontiguousarray(np.asarray(W2o, np.float32).astype(bf)),
        "f": np.ascontiguousarray(np.asarray(W2f, np.float32).astype(bf)),
    }
    b1 = {
        "o": np.ascontiguousarray(np.asarray(b1o, np.float32)),
        "f": np.ascontiguousarray(np.asarray(b1f, np.float32)),
    }
    b2 = {
        "o": np.ascontiguousarray(np.asarray(b2o, np.float32)),
        "f": np.ascontiguousarray(np.asarray(b2f, np.float32)),
    }
    xb = x.astype(bf)

    nc = _get_module()

    in_maps = []
    for i in range(N_CORES):
        shard = np.ascontiguousarray(xb[i * M : (i + 1) * M, :].T)
        m = {"xT": shard}
        for n in ("o", "f"):
            m[f"w1{n}"] = w1[n]
            m[f"w2{n}"] = w2[n]
            m[f"b1{n}"] = b1[n]
            m[f"b2{n}"] = b2[n]
        in_maps.append(m)

    trace = bool(os.environ.get("KERNEL_TRACE"))
    results = run_bass_kernel_spmd(
        nc, in_maps, list(range(N_CORES)), trace=trace
    )
    global LAST_RESULTS
    LAST_RESULTS = results

    out = np.concatenate(
        [np.asarray(results.results[i]["out"], np.float32) for i in range(N_CORES)],
        axis=0,
    )
    return out
